# revision 21
# baseline (speedup 1.0000x reference)
"""GQA attention (llama-style, RoPE, causal) on 8 Trainium2 NeuronCores.

Problem: B=2, S=2048, DIM=2048, 16 q-heads / 4 kv-heads, head_dim=128.

Sharding: batch x kv-group. Core c handles batch b=c//4 and kv-group
g=c%4 (q-heads 4g..4g+3, kv-head g). Each core computes its 4 heads'
attention and a partial output projection against wo[:, 512g:512(g+1)];
the host sums the 4 partials per batch. No cross-core communication.

Device-side layout is fully "transposed": activations live as [dim, seq]
so every matmul's contraction dim sits on the SBUF partition axis:
  qT/kT     = W^T-chunks @ xT           [head_dim, S]      (PSUM accum over din)
  rope      = qT*cos2 + (P@qT)*sin2     (partition-pair swap via a
                                         128x128 permutation matmul)
  scoresT   = kT-block^T-free @ qT      [kpos 128, q 512]
  expT      = Exp(scoresT * 1/sqrt(d))  (ScalarE, PSUM->SBUF, fp16)
  E         = sum_kb expT               (DVE 4x fp16 accumulate)
  out^T     = sum_kb V-block @ expT     [head_dim, q]      (PSUM accum)
  sums      = allones^T @ E             [128, q]  (ONE matmul per head and
                                         chunk; the all-ones [128,128]
                                         stationary replicates the column
                                         sums across partitions, so 1/sums
                                         needs no separate broadcast)
  yT        = woT-chunks @ (out^T / sums)                  (PSUM accum)

Changes vs the 342us fp32r baseline:
- fp16 datapath end-to-end (weights/x/kT/vnat/exp/ot/y). Same 1 cyc/row
  PE rate as fp32r>=256, but halves all DMA (43MB -> ~22MB: faster ramp
  + store drain), enables DVE 2x/4x perf modes, and removes the fp32r
  4 cyc/row penalty on <256-wide (narrowed diagonal) matmuls.
- softmax column sums no longer burn a [1,512] matmul per k-block
  (160 x 512cyc): exp tiles accumulate elementwise into E on DVE
  (scalar_tensor_tensor, 4x mode at fp16), one ones^T@E matmul per
  (head, s-chunk). 1/sums = exp(-ln(s)) on ACT (ln/exp/copy share one
  activation table; nc.vector.reciprocal is a ~3.3us microcoded DVE op
  that stalls the whole norm chain).
- causal diagonal blocks narrowed: scores/exp/pv only cover columns
  >= block offset (saves 2x12288 PE cycles + ACT/DVE width).
  pv accumulation uses skip_group_check (narrow stop leaves sim-side
  zero-region flags; hardware per-element accumulate is exact).
- emission interleaves independent dense matmuls between attention
  block matmuls (filler-queue of generators) so the in-order PE queue
  never stalls on the ScalarE exp latency. Fill order per chunk: this
  chunk's q1-3 projections, next chunk's k/v/q0, then deferred output
  projections (sc3's 64-block attention absorbs yproj(1)+yproj(2)).
- PSUM y evictions split ACT-half + DVE-half so the 2-bank y rotation
  outruns the 4-matmul accumulate per dm block; the FINAL chunk's yproj
  pipelines h0-2 accumulation one dm ahead of h3+evict over 3 banks
  (pacc is projection-free by then), hiding its own norm latency and
  eviction waits; PE warmup runs on a memset tile (no DMA wait)
  bridging the ~10us DMA spin-up.
Measured ~253-257 us/core at full clock (2.4GHz, 216ns/512-wide matmul
issue) vs 342 us baseline; the chip's alternate ~2.0GHz P0 power state
shows as ~300+us runs, and duty-cycle util-throttling adds 6-20%
run-to-run noise either way. PE-active ~226us of which ~203us is the
matmul-cycle floor (487k cycles); the rest is ramp/tail/throttle.
"""

import numpy as np
from collections import deque
from contextlib import ExitStack

import bass_rust
import concourse.bass as bass
import concourse.mybir as mybir
import concourse.tile as tile
from concourse.bass_utils import run_bass_kernel_spmd

P = 128          # SBUF partitions / head_dim
S = 2048         # sequence length
D = 2048         # model dim
KC = 16          # contraction chunks of 128 over D
SC = 4           # s-chunks of 512
QW = 512         # moving-operand width
NH = 4           # q-heads per core
N_CORES = 8
SCALE = float(1.0 / np.sqrt(np.float32(128.0)))
F32 = mybir.dt.float32
F32R = mybir.dt.float32r
F16 = mybir.dt.float16
EXP = mybir.ActivationFunctionType.Exp
LN = mybir.ActivationFunctionType.Ln
MULT = mybir.AluOpType.mult
ADD = mybir.AluOpType.add


class _TC(tile.TileContext):
    """TileContext whose tail drain splits its semaphore waits into
    separate wait instructions — the walrus build here rejects a Drain
    carrying more than a couple of inline sync waits."""

    def _drain_and_barrier(self, tick_clock, wait_clock):
        gc = tick_clock.global_clock
        ticks = [gc[i] for i in range(27)]
        for proc, sem in sorted(self.sems.allocated().items()):
            t = ticks[proc]
            if t > 0:
                mult = 16 if sem.name.startswith(("DMAHW", "DMASW")) else 1
                self.nc.sync.wait_ge(sem, t * mult)
        self.nc.sync.drain()
        self.nc.all_engine_barrier()
        popped = self.nc._tile_sem_poison_stack.pop()
        assert popped is self._sem_poison
        self.nc.clear_and_free_semaphores(list(self.sems.allocated().values()))
        self.nc.all_engine_barrier()


def _split_excess_waits(nc, max_waits=1):
    """This walrus build allows very few inline sync waits per TPB
    instruction (the fp32r self-loading Matmult takes only one). Move
    excess waits onto injected same-engine NOPs placed just before the
    instruction — semantically identical, since the engine queue executes
    in order."""
    for f in nc.m.functions:
        for blk in f.blocks:
            insts = blk.instructions
            new_list = []
            for inst in insts:
                si = inst.sync_info
                if si is not None and len(si.on_wait) > max_waits:
                    waits = list(si.on_wait)
                    excess, keep = waits[:-max_waits], waits[-max_waits:]
                    for j, w in enumerate(excess):
                        nop = bass_rust.InstNoOp(name=f"{inst.name}-wn{j}")
                        nop.engine = inst.engine
                        nop.sync_info = bass_rust.SyncInfo(
                            on_wait=[w], on_update=[])
                        new_list.append(nop)
                    inst.sync_info = bass_rust.SyncInfo(
                        on_wait=keep, on_update=list(si.on_update))
                new_list.append(inst)
            insts[:] = new_list


def _emit(nc, tc, ctx, t):
    pool = lambda name, bufs, space="SBUF": ctx.enter_context(
        tc.tile_pool(name=name, bufs=bufs, space=space)
    )

    # SBUF pools
    constp = pool("constp", 1)  # weights, x, trig tables, masks, k/v slabs
    qsbp = pool("qsbp", 2)     # pre-rope proj copy
    t1p = pool("t1p", 2)
    t2p = pool("t2p", 2)
    qrp = pool("qrp", 5)       # rope'd q tiles
    vsbp = pool("vsbp", 2)     # pre-transpose v copy
    ep = pool("ep", 3)         # exp tiles
    ebp = pool("ebp", 2)       # exp accumulators E
    rp = pool("rp", 2)         # reciprocal [1, 512]
    rbp = pool("rbp", 2)       # broadcast recip [128, 512]
    otp = pool("otp", 14)      # normalized attention out
    yp = pool("yp", 2)         # output copy slabs [128, 1024]

    # PSUM pools — 8 banks total
    pacc = pool("pacc", 2, "PSUM")    # proj accumulators      (2 banks)
    ptmp = pool("ptmp", 1, "PSUM")    # rope swap / v transpose / bcast / y
    pscore = pool("pscore", 2, "PSUM")  # scoresT              (2)
    pout = pool("pout", 2, "PSUM")    # attention out accum    (2)
    psmp = pool("psmp", 1, "PSUM")    # exp sums [1,512] / y   (1)

    # resident SBUF slabs (all fp16)
    x_sb = constp.tile([P, KC * S], F16, tag="x")       # chunk k at k*2048
    wq_sb = constp.tile([P, KC * 4 * P], F16, tag="wq")  # (h,k) at h*2048+k*128
    wk_sb = constp.tile([P, KC * P], F16, tag="wk")      # chunk k at k*128
    wv_sb = constp.tile([P, KC * P], F16, tag="wv")
    wo_sb = constp.tile([P, NH * S], F16, tag="wo")      # (h,dm) at h*2048+dm*128
    cos_sb = constp.tile([P, S], F16, tag="cos")
    sin_sb = constp.tile([P, S], F16, tag="sin")
    tri_sb = constp.tile([P, P], F16, tag="tri")         # tri[p,c]=1 iff c>=p
    perm_sb = constp.tile([P, P], F16, tag="perm")       # pair-swap permutation
    ident_sb = constp.tile([P, P], F16, tag="ident")
    ones_sb = constp.tile([P, P], F16, tag="ones")

    kT_sb = constp.tile([P, S], F16, tag="kT")    # rope'd K^T per s-chunk
    vnat_sb = constp.tile([P, S], F16, tag="vn")  # V natural [kpos, d]

    xT_d, yT_d = t["xT"], t["yT"]

    # ---- filler queue: generators that emit one PE matmul per next() ----
    filler = deque()

    def fill(n):
        k = 0
        while k < n and filler:
            try:
                next(filler[0])
                k += 1
            except StopIteration:
                filler.popleft()

    def drain_filler():
        while filler:
            try:
                next(filler[0])
            except StopIteration:
                filler.popleft()

    qr_tiles = {}

    def proj_gen(sc, pi):
        # pi: 0..3 = q heads, 4 = k, 5 = v
        ssl = slice(QW * sc, QW * (sc + 1))
        ps = pacc.tile([P, QW], F32, tag="acc")
        for k in range(KC):
            if pi < 4:
                w_ap = wq_sb[:, pi * 2048 + k * P:pi * 2048 + (k + 1) * P]
            elif pi == 4:
                w_ap = wk_sb[:, k * P:(k + 1) * P]
            else:
                w_ap = wv_sb[:, k * P:(k + 1) * P]
            nc.tensor.matmul(
                ps[:], w_ap, x_sb[:, S * k + QW * sc:S * k + QW * (sc + 1)],
                start=(k == 0), stop=(k == KC - 1),
            )
            yield
        if pi == 5:
            # V: psum -> sbuf fp16, then PE-transpose 128-blocks into vnat
            vsb = vsbp.tile([P, QW], F16, tag="vsb")
            nc.scalar.copy(vsb[:], ps[:])
            for tb in range(4):
                pt = ptmp.tile([P, P], F16, tag="tmp")
                nc.tensor.transpose(pt[:], vsb[:, P * tb:P * (tb + 1)], ident_sb[:])
                blk = 4 * sc + tb
                nc.scalar.copy(vnat_sb[:, P * blk:P * (blk + 1)], pt[:])
                yield
        else:
            # Q/K: rope = psum*cos2 + (perm @ psum)*sin2
            qsb = qsbp.tile([P, QW], F16, tag="qsb")
            nc.scalar.copy(qsb[:], ps[:])
            sw = ptmp.tile([P, QW], F32, tag="tmp")
            nc.tensor.matmul(sw[:], perm_sb[:], qsb[:], start=True, stop=True)
            yield
            t1 = t1p.tile([P, QW], F16, tag="t1")
            nc.vector.tensor_mul(t1[:], qsb[:], cos_sb[:, ssl])
            t2 = t2p.tile([P, QW], F16, tag="t2")
            nc.vector.tensor_mul(t2[:], sw[:], sin_sb[:, ssl])
            if pi < 4:
                dst = qrp.tile([P, QW], F16, tag="qr")
                qr_tiles[(sc, pi)] = dst
                nc.vector.tensor_add(dst[:], t1[:], t2[:])
            else:
                nc.vector.tensor_add(kT_sb[:, ssl], t1[:], t2[:])

    o_tiles = {}
    norm_pending = []

    def _emit_norm(po, rb, sc, h):
        ot = otp.tile([P, QW], F16, tag="ot")
        o_tiles[(sc, h)] = ot
        nc.vector.tensor_mul(ot[:], po[:], rb[:])

    def attn_head(sc, h):
        while (sc, h) not in qr_tiles and filler:
            fill(1)
        qr = qr_tiles.pop((sc, h))
        nkb = 4 * sc + 4
        po = pout.tile([P, QW], F32, tag="out")
        E = ebp.tile([P, QW], F16, tag="E")
        prev = None
        for kb in range(nkb):
            off = P * kb - QW * sc
            lo = max(off, 0)
            psc = pscore.tile([P, QW], F32, tag="score")
            nc.tensor.matmul(
                psc[:, lo:], kT_sb[:, P * kb:P * (kb + 1)], qr[:, lo:],
                start=True, stop=True,
            )
            et = ep.tile([P, QW], F16, tag="exp")
            nc.scalar.activation(et[:, lo:], psc[:, lo:], EXP, scale=SCALE)
            if off >= 0:  # diagonal block: zero kpos > q on a 128-wide strip
                nc.vector.tensor_mul(
                    et[:, lo:lo + P], et[:, lo:lo + P], tri_sb[:])
            if kb == 0:
                nc.vector.tensor_copy(E[:], et[:])
            else:
                nc.vector.scalar_tensor_tensor(
                    E[:, lo:], et[:, lo:], 1.0, E[:, lo:], op0=MULT, op1=ADD)
            if kb == 1 and norm_pending:
                _emit_norm(*norm_pending.pop())
            if prev is not None:
                pkb, plo, pet = prev
                nc.tensor.matmul(
                    po[:, plo:], vnat_sb[:, P * pkb:P * (pkb + 1)], pet[:, plo:],
                    start=(pkb == 0), stop=False, skip_group_check=True,
                )
            prev = (kb, lo, et)
            fill(3)
        pkb, plo, pet = prev
        nc.tensor.matmul(
            po[:, plo:], vnat_sb[:, P * pkb:P * (pkb + 1)], pet[:, plo:],
            start=(pkb == 0), stop=True, skip_group_check=True,
        )
        fill(2)
        # all-ones [128,128] stationary: ones^T @ E = column sums REPLICATED
        # on every output partition — the softmax denominator arrives already
        # broadcast, same 512-cycle cost as a [1,512] sums matmul
        psm = psmp.tile([P, QW], F32, tag="sum")
        nc.tensor.matmul(psm[:], ones_sb[:], E[:], start=True, stop=True)
        # 1/sums = exp(-ln(sums)) on ACT: ln/exp/copy share one activation
        # table (no reloads) and the DVE reciprocal is a ~3.3us microcoded op
        # that stalls the whole norm chain
        lnr = rp.tile([P, QW], F32, tag="r32")
        nc.scalar.activation(lnr[:], psm[:], LN)
        rb = rbp.tile([P, QW], F16, tag="rb")
        nc.scalar.activation(rb[:], lnr[:], EXP, scale=-1.0)
        norm_pending.append((po, rb, sc, h))
        fill(1)

    def yproj_gen(sc):
        ssl = slice(QW * sc, QW * (sc + 1))
        ysb = None

        def _evict(dm, py):
            nonlocal ysb
            if dm % 2 == 0:
                ysb = yp.tile([P, 2 * QW], F16, tag="y")
                lo = 0
            else:
                lo = QW
            nc.scalar.copy(ysb[:, lo:lo + QW // 2], py[:, :QW // 2])
            nc.vector.tensor_copy(ysb[:, lo + QW // 2:lo + QW], py[:, QW // 2:])
            if dm % 2 == 1:
                for j in range(2):
                    dmj = dm - 1 + j
                    nc.sync.dma_start(
                        yT_d[P * dmj:P * (dmj + 1), ssl],
                        ysb[:, QW * j:QW * (j + 1)])

        def _wo(h, dm):
            return wo_sb[:, S * h + P * dm:S * h + P * (dm + 1)]

        if sc == 3:
            # final chunk: runs right after its own norm chain with an empty
            # filler queue. Pipeline 2 stages over 3 banks (pacc is free of
            # projections): each dm's h0-2 accumulate one dm AHEAD of its
            # h3 matmul, so the first ot3-dependent matmul lands ~6 matmuls
            # after emission while the norm ACT chain completes.
            open_dm = None
            for dm in range(KC + 1):
                if dm < KC:
                    py = (ptmp, psmp, pacc)[dm % 3].tile(
                        [P, QW], F32, tag=("tmp", "sum", "acc")[dm % 3])
                    for h in range(3):
                        nc.tensor.matmul(py[:], _wo(h, dm),
                                         o_tiles[(sc, h)][:],
                                         start=(h == 0), stop=False)
                        yield
                    nxt = (dm, py)
                else:
                    nxt = None
                if open_dm is not None:
                    dmc, pyc = open_dm
                    nc.tensor.matmul(pyc[:], _wo(3, dmc),
                                     o_tiles[(sc, 3)][:],
                                     start=False, stop=True)
                    yield
                    _evict(dmc, pyc)
                open_dm = nxt
        else:
            for dm in range(KC):
                py = (ptmp, psmp)[dm % 2].tile(
                    [P, QW], F32, tag=("tmp", "sum")[dm % 2])
                for h in range(NH):
                    nc.tensor.matmul(
                        py[:], _wo(h, dm), o_tiles[(sc, h)][:],
                        start=(h == 0), stop=(h == NH - 1),
                    )
                    yield
                _evict(dm, py)
        for h in range(NH):
            del o_tiles[(sc, h)]

    # ---- prologue: DMAs + PE warmup ----
    # tiny tensors first; junk matmuls on them heat the PE clock (HAM)
    # while the big DMA stream ramps
    nc.sync.dma_start(perm_sb[:], t["perm"][:])
    nc.sync.dma_start(ident_sb[:], t["ident"][:])
    nc.sync.dma_start(tri_sb[:], t["tri"][:])
    nc.sync.dma_start(ones_sb[:], t["onescol"][:])
    wz = constp.tile([P, P], F16, tag="wz")
    nc.gpsimd.memset(wz[:], 0)
    wup = ptmp.tile([P, P], F32, tag="tmp")
    for _ in range(55):
        nc.tensor.matmul(wup[:], wz[:], wz[:], start=True, stop=True)
    # wk/wv lead (K,V projections run first), then x chunks in consumption
    # order, ~256KB per DMA to spread across the 8 HWDGE queues
    for half in range(2):
        hs = slice(P * KC // 2 * half, P * KC // 2 * (half + 1))
        nc.sync.dma_start(wk_sb[:, hs], t["wk"][:, hs])
        nc.sync.dma_start(wv_sb[:, hs], t["wv"][:, hs])

    def _x_half(k, half):
        lo = S * k + (S // 2) * half
        nc.sync.dma_start(
            x_sb[:, lo:lo + S // 2],
            xT_d[P * k:P * (k + 1), (S // 2) * half:(S // 2) * (half + 1)])

    # x first halves feed sc=0/1; interleave wq quarters (needed once the
    # q0 projection starts ~10us in); x second halves (sc=2/3) go last
    for k in range(4):
        lo = S * k
        for q in range(2):
            nc.sync.dma_start(
                x_sb[:, lo + 512 * q:lo + 512 * (q + 1)],
                xT_d[P * k:P * (k + 1), 512 * q:512 * (q + 1)])
    for k in range(4, 8):
        _x_half(k, 0)
    for q in range(2):
        nc.sync.dma_start(wq_sb[:, 1024 * q:1024 * (q + 1)],
                          t["wq"][:, 1024 * q:1024 * (q + 1)])
    for k in range(8, KC):
        _x_half(k, 0)
    nc.sync.dma_start(cos_sb[:], t["cos2"][:])
    nc.sync.dma_start(sin_sb[:], t["sin2"][:])

    for q in range(2, 8):
        nc.sync.dma_start(wq_sb[:, 1024 * q:1024 * (q + 1)],
                          t["wq"][:, 1024 * q:1024 * (q + 1)])
    for h in range(NH):
        for half in range(2):
            lo = S * h + (S // 2) * half
            nc.sync.dma_start(wo_sb[:, lo:lo + S // 2],
                              t["woT"][P * h:P * (h + 1),
                                       (S // 2) * half:(S // 2) * (half + 1)])
    for k in range(KC):
        _x_half(k, 1)

    # sc=0 K, V, Q0 projections run inline (nothing to overlap them with)
    for pi in (4, 5, 0):
        for _ in proj_gen(0, pi):
            pass

    # deferred output projections: chunk sc's yproj runs as filler inside a
    # LATER chunk's attention. sc=3 has the most attention blocks and the
    # least projection work left, so it absorbs two deferred yprojs.
    y_release = {2: [0], 3: [1, 2]}
    pending_ys = {}
    for sc in range(SC):
        # dense work that overlaps this chunk's attention: the remaining
        # q projections, the NEXT chunk's k/v/q0 projections, then
        # deferred output projections (fill the attention tail)
        for pi in (1, 2, 3):
            filler.append(proj_gen(sc, pi))
        if sc + 1 < SC:
            for pi in (4, 5, 0):
                filler.append(proj_gen(sc + 1, pi))
        for psc_ in y_release.get(sc, ()):
            filler.append(pending_ys.pop(psc_))
        for h in range(NH):
            attn_head(sc, h)
        fill(4)
        _emit_norm(*norm_pending.pop())
        pending_ys[sc] = yproj_gen(sc)
    filler.append(pending_ys.pop(3))
    drain_filler()


def build():
    nc = bass.Bass("TRN2", target_bir_lowering=False, debug=False,
                   num_devices=N_CORES)
    t = {
        "xT": nc.dram_tensor("xT", [D, S], F16, kind="ExternalInput"),
        "wq": nc.dram_tensor("wq", [P, KC * 4 * P], F16, kind="ExternalInput"),
        "wk": nc.dram_tensor("wk", [P, KC * P], F16, kind="ExternalInput"),
        "wv": nc.dram_tensor("wv", [P, KC * P], F16, kind="ExternalInput"),
        "woT": nc.dram_tensor("woT", [NH * P, S], F16, kind="ExternalInput"),
        "cos2": nc.dram_tensor("cos2", [P, S], F16, kind="ExternalInput"),
        "sin2": nc.dram_tensor("sin2", [P, S], F16, kind="ExternalInput"),
        "tri": nc.dram_tensor("tri", [P, P], F16, kind="ExternalInput"),
        "perm": nc.dram_tensor("perm", [P, P], F16, kind="ExternalInput"),
        "ident": nc.dram_tensor("ident", [P, P], F16, kind="ExternalInput"),
        "onescol": nc.dram_tensor("onescol", [P, P], F16, kind="ExternalInput"),
        "yT": nc.dram_tensor("yT", [D, S], F16, kind="ExternalOutput"),
    }
    aps = {k: v.ap() for k, v in t.items()}
    with _TC(nc, num_cores=N_CORES) as tc:
        with ExitStack() as ctx:
            _emit(nc, tc, ctx, aps)
    _split_excess_waits(nc)
    return nc


def host_inputs(x, wq, wk, wv, wo, freqs_cos, freqs_sin):
    """Shard + repack the full inputs into per-core in_maps."""
    f16 = np.float16
    cos2 = np.repeat(np.ascontiguousarray(freqs_cos.T), 2, axis=0).astype(f16)
    sin_t = np.ascontiguousarray(freqs_sin.T).astype(np.float32)
    sin2 = np.empty((P, S), np.float32)
    sin2[0::2] = -sin_t
    sin2[1::2] = sin_t
    sin2 = sin2.astype(f16)
    pidx = np.arange(P)
    tri = (np.arange(P)[None, :] >= pidx[:, None]).astype(f16)
    perm = np.zeros((P, P), np.float32)
    perm[pidx, pidx ^ 1] = 1.0
    perm = perm.astype(f16)
    ident = np.eye(P, dtype=f16)

    in_maps = []
    for c in range(N_CORES):
        b, g = divmod(c, 4)
        xT = np.ascontiguousarray(x[b].T).astype(f16)
        wq_s = wq[512 * g:512 * (g + 1)]                  # [512, 2048]
        wq_r = np.ascontiguousarray(
            wq_s.reshape(4, P, KC, P).transpose(3, 0, 2, 1).reshape(P, KC * 4 * P)
        ).astype(f16)
        wk_s = wk[P * g:P * (g + 1)]                      # [128, 2048]
        wk_r = np.ascontiguousarray(
            wk_s.reshape(P, KC, P).transpose(2, 1, 0).reshape(P, KC * P)
        ).astype(f16)
        wv_s = wv[P * g:P * (g + 1)]
        wv_r = np.ascontiguousarray(
            wv_s.reshape(P, KC, P).transpose(2, 1, 0).reshape(P, KC * P)
        ).astype(f16)
        woT = np.ascontiguousarray(wo[:, 512 * g:512 * (g + 1)].T).astype(f16)
        in_maps.append({
            "xT": xT, "wq": wq_r, "wk": wk_r, "wv": wv_r, "woT": woT,
            "cos2": cos2, "sin2": sin2, "tri": tri, "perm": perm,
            "ident": ident,
            "onescol": np.ones((P, P), f16),
        })
    return in_maps


def combine_outputs(results):
    out = np.empty((2, S, D), np.float32)
    for b in range(2):
        acc = results[4 * b]["yT"].astype(np.float32)
        for g in range(1, 4):
            acc += results[4 * b + g]["yT"].astype(np.float32)
        out[b] = acc.T
    return out


_NC_CACHE = []


def kernel(x, wq, wk, wv, wo, freqs_cos, freqs_sin, mask):
    del mask  # causal structure handled on-device
    if not _NC_CACHE:
        _NC_CACHE.append(build())
    nc = _NC_CACHE[0]
    in_maps = host_inputs(x, wq, wk, wv, wo, freqs_cos, freqs_sin)
    res = run_bass_kernel_spmd(nc, in_maps, list(range(N_CORES)))
    return combine_outputs(res.results)


# revision 22
# speedup vs baseline: 1.1833x; 1.1833x over previous
"""GQA attention (llama-style, RoPE, causal) on 8 Trainium2 NeuronCores.

Problem: B=2, S=2048, DIM=2048, 16 q-heads / 4 kv-heads, head_dim=128.

Sharding: batch x kv-group. Core c handles batch b=c//4 and kv-group
g=c%4 (q-heads 4g..4g+3, kv-head g). Each core computes its 4 heads'
attention and a partial output projection against wo[:, 512g:512(g+1)];
the host sums the 4 partials per batch. No cross-core communication.

Device-side layout is fully "transposed": activations live as [dim, seq]
so every matmul's contraction dim sits on the SBUF partition axis:
  qT/kT     = W^T-chunks @ xT           [head_dim, S]      (PSUM accum over din)
  rope      = qT*cos2 + (P@qT)*sin2     (partition-pair swap via a
                                         128x128 permutation matmul)
  scoresT   = kT-block^T-free @ qT      [kpos 128, q 512]
  expT      = Exp(scoresT * 1/sqrt(d))  (ScalarE, PSUM->SBUF, fp16)
  E         = sum_kb expT               (DVE 4x fp16 accumulate)
  out^T     = sum_kb V-block @ expT     [head_dim, q]      (PSUM accum)
  sums      = allones^T @ E             [128, q]  (ONE matmul per head and
                                         chunk; the all-ones [128,128]
                                         stationary replicates the column
                                         sums across partitions, so 1/sums
                                         needs no separate broadcast)
  yT        = woT-chunks @ (out^T / sums)                  (PSUM accum)

Changes vs the 342us fp32r baseline:
- fp16 datapath end-to-end (weights/x/kT/vnat/exp/ot/y). Same 1 cyc/row
  PE rate as fp32r>=256, but halves all DMA (43MB -> ~22MB: faster ramp
  + store drain), enables DVE 2x/4x perf modes, and removes the fp32r
  4 cyc/row penalty on <256-wide (narrowed diagonal) matmuls.
- softmax column sums no longer burn a [1,512] matmul per k-block
  (160 x 512cyc): exp tiles accumulate elementwise into E on DVE
  (scalar_tensor_tensor, 4x mode at fp16), one ones^T@E matmul per
  (head, s-chunk). 1/sums = exp(-ln(s)) on ACT (ln/exp/copy share one
  activation table; nc.vector.reciprocal is a ~3.3us microcoded DVE op
  that stalls the whole norm chain).
- causal diagonal blocks narrowed: scores/exp/pv only cover columns
  >= block offset (saves 2x12288 PE cycles + ACT/DVE width).
  pv accumulation uses skip_group_check (narrow stop leaves sim-side
  zero-region flags; hardware per-element accumulate is exact).
- emission interleaves independent dense matmuls between attention
  block matmuls (filler-queue of generators) so the in-order PE queue
  never stalls on the ScalarE exp latency. Fill order per chunk: this
  chunk's q1-3 projections, next chunk's k/v/q0, then deferred output
  projections (sc3's 64-block attention absorbs yproj(1)+yproj(2)).
- PSUM y evictions split ACT-half + DVE-half so the 2-bank y rotation
  outruns the 4-matmul accumulate per dm block; the FINAL chunk's yproj
  pipelines h0-2 accumulation one dm ahead of h3+evict over 3 banks
  (pacc is projection-free by then), hiding its own norm latency and
  eviction waits; PE warmup runs on a memset tile (no DMA wait)
  bridging the ~10us DMA spin-up.
Measured ~253-257 us/core at full clock (2.4GHz, 216ns/512-wide matmul
issue) vs 342 us baseline; the chip's alternate ~2.0GHz P0 power state
shows as ~300+us runs, and duty-cycle util-throttling adds 6-20%
run-to-run noise either way. PE-active ~226us of which ~203us is the
matmul-cycle floor (487k cycles); the rest is ramp/tail/throttle.
"""

import numpy as np
from collections import deque
from contextlib import ExitStack

import bass_rust
import concourse.bass as bass
import concourse.mybir as mybir
import concourse.tile as tile
from concourse.bass_utils import run_bass_kernel_spmd

P = 128          # SBUF partitions / head_dim
S = 2048         # sequence length
D = 2048         # model dim
KC = 16          # contraction chunks of 128 over D
SC = 4           # s-chunks of 512
QW = 512         # moving-operand width
NH = 4           # q-heads per core
N_CORES = 8
SCALE = float(1.0 / np.sqrt(np.float32(128.0)))
F32 = mybir.dt.float32
F32R = mybir.dt.float32r
F16 = mybir.dt.float16
EXP = mybir.ActivationFunctionType.Exp
LN = mybir.ActivationFunctionType.Ln
MULT = mybir.AluOpType.mult
ADD = mybir.AluOpType.add


class _TC(tile.TileContext):
    """TileContext whose tail drain splits its semaphore waits into
    separate wait instructions — the walrus build here rejects a Drain
    carrying more than a couple of inline sync waits."""

    def _drain_and_barrier(self, tick_clock, wait_clock):
        gc = tick_clock.global_clock
        ticks = [gc[i] for i in range(27)]
        for proc, sem in sorted(self.sems.allocated().items()):
            t = ticks[proc]
            if t > 0:
                mult = 16 if sem.name.startswith(("DMAHW", "DMASW")) else 1
                self.nc.sync.wait_ge(sem, t * mult)
        self.nc.sync.drain()
        self.nc.all_engine_barrier()
        popped = self.nc._tile_sem_poison_stack.pop()
        assert popped is self._sem_poison
        self.nc.clear_and_free_semaphores(list(self.sems.allocated().values()))
        self.nc.all_engine_barrier()


def _split_excess_waits(nc, max_waits=1):
    """This walrus build allows very few inline sync waits per TPB
    instruction (the fp32r self-loading Matmult takes only one). Move
    excess waits onto injected same-engine NOPs placed just before the
    instruction — semantically identical, since the engine queue executes
    in order."""
    for f in nc.m.functions:
        for blk in f.blocks:
            insts = blk.instructions
            new_list = []
            for inst in insts:
                si = inst.sync_info
                if si is not None and len(si.on_wait) > max_waits:
                    waits = list(si.on_wait)
                    excess, keep = waits[:-max_waits], waits[-max_waits:]
                    for j, w in enumerate(excess):
                        nop = bass_rust.InstNoOp(name=f"{inst.name}-wn{j}")
                        nop.engine = inst.engine
                        nop.sync_info = bass_rust.SyncInfo(
                            on_wait=[w], on_update=[])
                        new_list.append(nop)
                    inst.sync_info = bass_rust.SyncInfo(
                        on_wait=keep, on_update=list(si.on_update))
                new_list.append(inst)
            insts[:] = new_list


def _emit(nc, tc, ctx, t):
    pool = lambda name, bufs, space="SBUF": ctx.enter_context(
        tc.tile_pool(name=name, bufs=bufs, space=space)
    )

    # SBUF pools
    constp = pool("constp", 1)  # weights, x, trig tables, masks, k/v slabs
    qsbp = pool("qsbp", 2)     # pre-rope proj copy
    t1p = pool("t1p", 2)
    t2p = pool("t2p", 2)
    qrp = pool("qrp", 5)       # rope'd q tiles
    vsbp = pool("vsbp", 2)     # pre-transpose v copy
    ep = pool("ep", 3)         # exp tiles
    ebp = pool("ebp", 2)       # exp accumulators E
    rp = pool("rp", 2)         # reciprocal [1, 512]
    rbp = pool("rbp", 2)       # broadcast recip [128, 512]
    otp = pool("otp", 14)      # normalized attention out
    yp = pool("yp", 2)         # output copy slabs [128, 1024]

    # PSUM pools — 8 banks total
    pacc = pool("pacc", 2, "PSUM")    # proj accumulators      (2 banks)
    ptmp = pool("ptmp", 1, "PSUM")    # rope swap / v transpose / bcast / y
    pscore = pool("pscore", 2, "PSUM")  # scoresT              (2)
    pout = pool("pout", 2, "PSUM")    # attention out accum    (2)
    psmp = pool("psmp", 1, "PSUM")    # exp sums [1,512] / y   (1)

    # resident SBUF slabs (all fp16)
    x_sb = constp.tile([P, KC * S], F16, tag="x")       # chunk k at k*2048
    wq_sb = constp.tile([P, KC * 4 * P], F16, tag="wq")  # (h,k) at h*2048+k*128
    wk_sb = constp.tile([P, KC * P], F16, tag="wk")      # chunk k at k*128
    wv_sb = constp.tile([P, KC * P], F16, tag="wv")
    wo_sb = constp.tile([P, NH * S], F16, tag="wo")      # (h,dm) at h*2048+dm*128
    cos_sb = constp.tile([P, S], F16, tag="cos")
    sin_sb = constp.tile([P, S], F16, tag="sin")
    tri_sb = constp.tile([P, P], F16, tag="tri")         # tri[p,c]=1 iff c>=p
    perm_sb = constp.tile([P, P], F16, tag="perm")       # pair-swap permutation
    ident_sb = constp.tile([P, P], F16, tag="ident")
    ones_sb = constp.tile([P, P], F16, tag="ones")

    kT_sb = constp.tile([P, S], F16, tag="kT")    # rope'd K^T per s-chunk
    vnat_sb = constp.tile([P, S], F16, tag="vn")  # V natural [kpos, d]

    xT_d, yT_d = t["xT"], t["yT"]

    # ---- filler queue: generators that emit one PE matmul per next() ----
    filler = deque()

    def fill(n):
        k = 0
        while k < n and filler:
            try:
                next(filler[0])
                k += 1
            except StopIteration:
                filler.popleft()

    def drain_filler():
        while filler:
            try:
                next(filler[0])
            except StopIteration:
                filler.popleft()

    qr_tiles = {}

    def proj_gen(sc, pi, psum_pool=None, psum_tag="acc"):
        # pi: 0..3 = q heads, 4 = k, 5 = v
        ssl = slice(QW * sc, QW * (sc + 1))
        ps = (psum_pool or pacc).tile([P, QW], F32, tag=psum_tag)
        for k in range(KC):
            if pi < 4:
                w_ap = wq_sb[:, pi * 2048 + k * P:pi * 2048 + (k + 1) * P]
            elif pi == 4:
                w_ap = wk_sb[:, k * P:(k + 1) * P]
            else:
                w_ap = wv_sb[:, k * P:(k + 1) * P]
            nc.tensor.matmul(
                ps[:], w_ap, x_sb[:, S * k + QW * sc:S * k + QW * (sc + 1)],
                start=(k == 0), stop=(k == KC - 1),
            )
            yield
        if pi == 5:
            # V: psum -> sbuf fp16, then PE-transpose 128-blocks into vnat
            vsb = vsbp.tile([P, QW], F16, tag="vsb")
            nc.scalar.copy(vsb[:], ps[:])
            for tb in range(4):
                pt = ptmp.tile([P, P], F16, tag="tmp")
                nc.tensor.transpose(pt[:], vsb[:, P * tb:P * (tb + 1)], ident_sb[:])
                blk = 4 * sc + tb
                nc.scalar.copy(vnat_sb[:, P * blk:P * (blk + 1)], pt[:])
                yield
        else:
            # Q/K: rope = psum*cos2 + (perm @ psum)*sin2
            qsb = qsbp.tile([P, QW], F16, tag="qsb")
            nc.scalar.copy(qsb[:], ps[:])
            sw = ptmp.tile([P, QW], F32, tag="tmp")
            nc.tensor.matmul(sw[:], perm_sb[:], qsb[:], start=True, stop=True)
            yield
            t1 = t1p.tile([P, QW], F16, tag="t1")
            nc.vector.tensor_mul(t1[:], qsb[:], cos_sb[:, ssl])
            t2 = t2p.tile([P, QW], F16, tag="t2")
            nc.vector.tensor_mul(t2[:], sw[:], sin_sb[:, ssl])
            if pi < 4:
                dst = qrp.tile([P, QW], F16, tag="qr")
                qr_tiles[(sc, pi)] = dst
                nc.vector.tensor_add(dst[:], t1[:], t2[:])
            else:
                nc.vector.tensor_add(kT_sb[:, ssl], t1[:], t2[:])

    o_tiles = {}
    norm_pending = []

    def _emit_norm(po, rb, sc, h):
        ot = otp.tile([P, QW], F16, tag="ot")
        o_tiles[(sc, h)] = ot
        nc.vector.tensor_mul(ot[:], po[:], rb[:])

    def attn_head(sc, h):
        while (sc, h) not in qr_tiles and filler:
            fill(1)
        qr = qr_tiles.pop((sc, h))
        nkb = 4 * sc + 4
        po = pout.tile([P, QW], F32, tag="out")
        E = ebp.tile([P, QW], F16, tag="E")
        prev = None
        for kb in range(nkb):
            off = P * kb - QW * sc
            lo = max(off, 0)
            psc = pscore.tile([P, QW], F32, tag="score")
            nc.tensor.matmul(
                psc[:, lo:], kT_sb[:, P * kb:P * (kb + 1)], qr[:, lo:],
                start=True, stop=True,
            )
            et = ep.tile([P, QW], F16, tag="exp")
            nc.scalar.activation(et[:, lo:], psc[:, lo:], EXP, scale=SCALE)
            if off >= 0:  # diagonal block: zero kpos > q on a 128-wide strip
                nc.vector.tensor_mul(
                    et[:, lo:lo + P], et[:, lo:lo + P], tri_sb[:])
            if kb == 0:
                nc.vector.tensor_copy(E[:], et[:])
            else:
                nc.vector.scalar_tensor_tensor(
                    E[:, lo:], et[:, lo:], 1.0, E[:, lo:], op0=MULT, op1=ADD)
            if kb == 1 and norm_pending:
                _emit_norm(*norm_pending.pop())
            if prev is not None:
                pkb, plo, pet = prev
                nc.tensor.matmul(
                    po[:, plo:], vnat_sb[:, P * pkb:P * (pkb + 1)], pet[:, plo:],
                    start=(pkb == 0), stop=False, skip_group_check=True,
                )
            prev = (kb, lo, et)
            fill(3)
        pkb, plo, pet = prev
        nc.tensor.matmul(
            po[:, plo:], vnat_sb[:, P * pkb:P * (pkb + 1)], pet[:, plo:],
            start=(pkb == 0), stop=True, skip_group_check=True,
        )
        fill(2)
        # all-ones [128,128] stationary: ones^T @ E = column sums REPLICATED
        # on every output partition — the softmax denominator arrives already
        # broadcast, same 512-cycle cost as a [1,512] sums matmul
        psm = psmp.tile([P, QW], F32, tag="sum")
        nc.tensor.matmul(psm[:], ones_sb[:], E[:], start=True, stop=True)
        # 1/sums = exp(-ln(sums)) on ACT: ln/exp/copy share one activation
        # table (no reloads) and the DVE reciprocal is a ~3.3us microcoded op
        # that stalls the whole norm chain
        lnr = rp.tile([P, QW], F32, tag="r32")
        nc.scalar.activation(lnr[:], psm[:], LN)
        rb = rbp.tile([P, QW], F16, tag="rb")
        nc.scalar.activation(rb[:], lnr[:], EXP, scale=-1.0)
        norm_pending.append((po, rb, sc, h))
        fill(1)

    def yproj_gen(sc):
        ssl = slice(QW * sc, QW * (sc + 1))
        ysb = None

        def _evict(dm, py):
            nonlocal ysb
            if dm % 2 == 0:
                ysb = yp.tile([P, 2 * QW], F16, tag="y")
                lo = 0
            else:
                lo = QW
            nc.scalar.copy(ysb[:, lo:lo + QW // 2], py[:, :QW // 2])
            nc.vector.tensor_copy(ysb[:, lo + QW // 2:lo + QW], py[:, QW // 2:])
            if sc == 3:  # final chunk: store per-dm so the drain tail shrinks
                nc.sync.dma_start(yT_d[P * dm:P * (dm + 1), ssl],
                                  ysb[:, lo:lo + QW])
            elif dm % 2 == 1:
                for j in range(2):
                    dmj = dm - 1 + j
                    nc.sync.dma_start(
                        yT_d[P * dmj:P * (dmj + 1), ssl],
                        ysb[:, QW * j:QW * (j + 1)])

        def _wo(h, dm):
            return wo_sb[:, S * h + P * dm:S * h + P * (dm + 1)]

        if sc == 3:
            # final chunk: runs right after its own norm chain with an empty
            # filler queue. Pipeline 2 stages over 3 banks (pacc is free of
            # projections): each dm's h0-2 accumulate one dm AHEAD of its
            # h3 matmul, so the first ot3-dependent matmul lands ~6 matmuls
            # after emission while the norm ACT chain completes.
            open_dm = None
            for dm in range(KC + 1):
                if dm < KC:
                    py = (ptmp, psmp, pacc)[dm % 3].tile(
                        [P, QW], F32, tag=("tmp", "sum", "acc")[dm % 3])
                    for h in range(3):
                        nc.tensor.matmul(py[:], _wo(h, dm),
                                         o_tiles[(sc, h)][:],
                                         start=(h == 0), stop=False)
                        yield
                    nxt = (dm, py)
                else:
                    nxt = None
                if open_dm is not None:
                    dmc, pyc = open_dm
                    nc.tensor.matmul(pyc[:], _wo(3, dmc),
                                     o_tiles[(sc, 3)][:],
                                     start=False, stop=True)
                    yield
                    _evict(dmc, pyc)
                open_dm = nxt
        else:
            for dm in range(KC):
                py = (ptmp, psmp)[dm % 2].tile(
                    [P, QW], F32, tag=("tmp", "sum")[dm % 2])
                for h in range(NH):
                    nc.tensor.matmul(
                        py[:], _wo(h, dm), o_tiles[(sc, h)][:],
                        start=(h == 0), stop=(h == NH - 1),
                    )
                    yield
                _evict(dm, py)
        for h in range(NH):
            del o_tiles[(sc, h)]

    # ---- prologue: DMAs + PE warmup ----
    # tiny tensors first; junk matmuls on them heat the PE clock (HAM)
    # while the big DMA stream ramps
    nc.sync.dma_start(perm_sb[:], t["perm"][:])
    nc.sync.dma_start(ident_sb[:], t["ident"][:])
    nc.sync.dma_start(tri_sb[:], t["tri"][:])
    nc.sync.dma_start(ones_sb[:], t["onescol"][:])
    wz = constp.tile([P, P], F16, tag="wz")
    nc.gpsimd.memset(wz[:], 0)
    wup = ptmp.tile([P, P], F32, tag="tmp")
    for _ in range(85):
        nc.tensor.matmul(wup[:], wz[:], wz[:], start=True, stop=True)
    # wk/wv lead (K,V projections run first), then x chunks in consumption
    # order, ~256KB per DMA to spread across the 8 HWDGE queues
    for half in range(2):
        hs = slice(P * KC // 2 * half, P * KC // 2 * (half + 1))
        nc.sync.dma_start(wk_sb[:, hs], t["wk"][:, hs])
        nc.sync.dma_start(wv_sb[:, hs], t["wv"][:, hs])

    def _x_half(k, half):
        lo = S * k + (S // 2) * half
        nc.sync.dma_start(
            x_sb[:, lo:lo + S // 2],
            xT_d[P * k:P * (k + 1), (S // 2) * half:(S // 2) * (half + 1)])

    # x first halves feed sc=0/1; interleave wq quarters (needed once the
    # q0 projection starts ~10us in); x second halves (sc=2/3) go last
    for k in range(4):
        lo = S * k
        for q in range(2):
            nc.sync.dma_start(
                x_sb[:, lo + 512 * q:lo + 512 * (q + 1)],
                xT_d[P * k:P * (k + 1), 512 * q:512 * (q + 1)])
    for k in range(4, 8):
        _x_half(k, 0)
    for q in range(2):
        nc.sync.dma_start(wq_sb[:, 1024 * q:1024 * (q + 1)],
                          t["wq"][:, 1024 * q:1024 * (q + 1)])
    for k in range(8, KC):
        _x_half(k, 0)
    nc.sync.dma_start(cos_sb[:], t["cos2"][:])
    nc.sync.dma_start(sin_sb[:], t["sin2"][:])

    for q in range(2, 8):
        nc.sync.dma_start(wq_sb[:, 1024 * q:1024 * (q + 1)],
                          t["wq"][:, 1024 * q:1024 * (q + 1)])
    for h in range(NH):
        for half in range(2):
            lo = S * h + (S // 2) * half
            nc.sync.dma_start(wo_sb[:, lo:lo + S // 2],
                              t["woT"][P * h:P * (h + 1),
                                       (S // 2) * half:(S // 2) * (half + 1)])
    for k in range(KC):
        _x_half(k, 1)

    # sc=0 K, V, Q0 projections run inline, round-robin by contraction
    # chunk so each arriving x chunk feeds 3 matmuls (the DMA ramp is the
    # limiter here). They spread across pacc + the still-idle pscore/pout.
    pro = deque([proj_gen(0, 4), proj_gen(0, 5, pscore, "score"),
                 proj_gen(0, 0, pout, "out")])
    while pro:
        g = pro.popleft()
        try:
            next(g)
            pro.append(g)
        except StopIteration:
            pass

    # deferred output projections: chunk sc's yproj runs as filler inside a
    # LATER chunk's attention. sc=3 has the most attention blocks and the
    # least projection work left, so it absorbs two deferred yprojs.
    y_release = {2: [0], 3: [1, 2]}
    pending_ys = {}
    for sc in range(SC):
        # dense work that overlaps this chunk's attention: the remaining
        # q projections, the NEXT chunk's k/v/q0 projections, then
        # deferred output projections (fill the attention tail)
        for pi in (1, 2, 3):
            filler.append(proj_gen(sc, pi))
        if sc + 1 < SC:
            for pi in (4, 5, 0):
                filler.append(proj_gen(sc + 1, pi))
        for psc_ in y_release.get(sc, ()):
            filler.append(pending_ys.pop(psc_))
        for h in range(NH):
            attn_head(sc, h)
        fill(4)
        _emit_norm(*norm_pending.pop())
        pending_ys[sc] = yproj_gen(sc)
    filler.append(pending_ys.pop(3))
    drain_filler()


def build():
    nc = bass.Bass("TRN2", target_bir_lowering=False, debug=False,
                   num_devices=N_CORES)
    t = {
        "xT": nc.dram_tensor("xT", [D, S], F16, kind="ExternalInput"),
        "wq": nc.dram_tensor("wq", [P, KC * 4 * P], F16, kind="ExternalInput"),
        "wk": nc.dram_tensor("wk", [P, KC * P], F16, kind="ExternalInput"),
        "wv": nc.dram_tensor("wv", [P, KC * P], F16, kind="ExternalInput"),
        "woT": nc.dram_tensor("woT", [NH * P, S], F16, kind="ExternalInput"),
        "cos2": nc.dram_tensor("cos2", [P, S], F16, kind="ExternalInput"),
        "sin2": nc.dram_tensor("sin2", [P, S], F16, kind="ExternalInput"),
        "tri": nc.dram_tensor("tri", [P, P], F16, kind="ExternalInput"),
        "perm": nc.dram_tensor("perm", [P, P], F16, kind="ExternalInput"),
        "ident": nc.dram_tensor("ident", [P, P], F16, kind="ExternalInput"),
        "onescol": nc.dram_tensor("onescol", [P, P], F16, kind="ExternalInput"),
        "yT": nc.dram_tensor("yT", [D, S], F16, kind="ExternalOutput"),
    }
    aps = {k: v.ap() for k, v in t.items()}
    with _TC(nc, num_cores=N_CORES) as tc:
        with ExitStack() as ctx:
            _emit(nc, tc, ctx, aps)
    _split_excess_waits(nc)
    return nc


def host_inputs(x, wq, wk, wv, wo, freqs_cos, freqs_sin):
    """Shard + repack the full inputs into per-core in_maps."""
    f16 = np.float16
    cos2 = np.repeat(np.ascontiguousarray(freqs_cos.T), 2, axis=0).astype(f16)
    sin_t = np.ascontiguousarray(freqs_sin.T).astype(np.float32)
    sin2 = np.empty((P, S), np.float32)
    sin2[0::2] = -sin_t
    sin2[1::2] = sin_t
    sin2 = sin2.astype(f16)
    pidx = np.arange(P)
    tri = (np.arange(P)[None, :] >= pidx[:, None]).astype(f16)
    perm = np.zeros((P, P), np.float32)
    perm[pidx, pidx ^ 1] = 1.0
    perm = perm.astype(f16)
    ident = np.eye(P, dtype=f16)

    in_maps = []
    for c in range(N_CORES):
        b, g = divmod(c, 4)
        xT = np.ascontiguousarray(x[b].T).astype(f16)
        wq_s = wq[512 * g:512 * (g + 1)]                  # [512, 2048]
        wq_r = np.ascontiguousarray(
            wq_s.reshape(4, P, KC, P).transpose(3, 0, 2, 1).reshape(P, KC * 4 * P)
        ).astype(f16)
        wk_s = wk[P * g:P * (g + 1)]                      # [128, 2048]
        wk_r = np.ascontiguousarray(
            wk_s.reshape(P, KC, P).transpose(2, 1, 0).reshape(P, KC * P)
        ).astype(f16)
        wv_s = wv[P * g:P * (g + 1)]
        wv_r = np.ascontiguousarray(
            wv_s.reshape(P, KC, P).transpose(2, 1, 0).reshape(P, KC * P)
        ).astype(f16)
        woT = np.ascontiguousarray(wo[:, 512 * g:512 * (g + 1)].T).astype(f16)
        in_maps.append({
            "xT": xT, "wq": wq_r, "wk": wk_r, "wv": wv_r, "woT": woT,
            "cos2": cos2, "sin2": sin2, "tri": tri, "perm": perm,
            "ident": ident,
            "onescol": np.ones((P, P), f16),
        })
    return in_maps


def combine_outputs(results):
    out = np.empty((2, S, D), np.float32)
    for b in range(2):
        acc = results[4 * b]["yT"].astype(np.float32)
        for g in range(1, 4):
            acc += results[4 * b + g]["yT"].astype(np.float32)
        out[b] = acc.T
    return out


_NC_CACHE = []


def kernel(x, wq, wk, wv, wo, freqs_cos, freqs_sin, mask):
    del mask  # causal structure handled on-device
    if not _NC_CACHE:
        _NC_CACHE.append(build())
    nc = _NC_CACHE[0]
    in_maps = host_inputs(x, wq, wk, wv, wo, freqs_cos, freqs_sin)
    res = run_bass_kernel_spmd(nc, in_maps, list(range(N_CORES)))
    return combine_outputs(res.results)
